# revision 2
# baseline (speedup 1.0000x reference)
"""Trainium2 Bass kernel for nn_Atomistic (per-species linear + segment sum).

Math:  out[j] = sum_{atoms a with structural_indices[a]==j} X[a,:] @ W[species[a],:,0]

Device strategy (8 NeuronCores, data-parallel over atoms):
  * Atoms are processed in chunks of 128 (partition dim = atom). F consecutive
    chunks form a "window"; because structural_indices is sorted, a window's
    atoms span fewer than SW = M/8 segments (verified on the host; parameter
    ladder degrades F/M if ever violated).
  * Per atom, code = 8*(g - window_base) + species in [0, M).  A bf16 one-hot
    oh[a, m] = (code[a] == m) is built on the Vector engine with a batched
    is_equal against a repeated iota (one instruction per NCB chunks).
  * X is split on the host into bf16 hi + bf16 lo (exact to ~2^-17) and laid
    out as the matmul stationary operand XHL[a, d'] with d' in [0,128) =
    [hi(64) | lo(64)].  One self-loading matmul per chunk accumulates
        PS[d', m] += sum_a XHL[a, d'] * oh[a, m]
    into PSUM; QW windows share one PSUM tile.
  * Flush: U = PS * wtile on the Vector engine (wtile[d', m] = W[s(m), d'%64]
    applies the per-species weights on device), then one matmul with a ones
    vector reduces over d' into a persistent PSUM accumulator column per
    window pair.  A final small matmul folds species; the [16, npairs] result
    is DMA'd out and scattered into out[20000] on the host at the window bases.
Host does only index prep / dtype split / layout; all FLOP-carrying work on
the 512 MB X stream happens on device.
"""
import sys

sys.path.insert(0, "/opt/trn_rl_repo")

import numpy as np
import ml_dtypes

N_ATOMS = 2_000_000
D_FEAT = 64
OUT_DIM = 1
N_SPECIES = 8
N_STRUCTURES = 20_000
N_CORES = 8

f32 = None
bf16 = None
_cache = {}


def _imports():
    global f32, bf16
    import concourse.mybir as mybir
    f32 = mybir.dt.float32
    bf16 = mybir.dt.bfloat16


def _build_program(M, F, NCB, QW, nch, nrep=1, n_cores=N_CORES,
                   xh_bufs=3, oh_bufs=3, ps_bufs=3, u_bufs=3):
    import concourse.mybir as mybir
    from concourse import tile, bacc
    _imports()
    assert nch % NCB == 0 and NCB % (QW * F) == 0
    nwin = nch // F
    npair = nwin // 2
    nc = bacc.Bacc("TRN2", target_bir_lowering=False, debug=False, num_devices=n_cores)
    xh = nc.dram_tensor("xh", [128, nch * 128], bf16, kind="ExternalInput").ap()
    code = nc.dram_tensor("code", [128, nch], f32, kind="ExternalInput").ap()
    iota = nc.dram_tensor("iota", [128, NCB * M], bf16, kind="ExternalInput").ap()
    wtile = nc.dram_tensor("wtile", [128, QW * M], f32, kind="ExternalInput").ap()
    ones1 = nc.dram_tensor("ones1", [128, 1], f32, kind="ExternalInput").ap()
    g8 = nc.dram_tensor("g8", [2 * M, 16], f32, kind="ExternalInput").ap()
    r = nc.dram_tensor("r", [16, npair], f32, kind="ExternalOutput").ap()

    with tile.TileContext(nc) as tc:
        with tc.tile_pool(name="const", bufs=1) as cp, \
             tc.tile_pool(name="xhp", bufs=xh_bufs) as xhp, \
             tc.tile_pool(name="ohp", bufs=oh_bufs) as ohp, \
             tc.tile_pool(name="usb", bufs=u_bufs) as usbp, \
             tc.tile_pool(name="psp", bufs=ps_bufs, space="PSUM") as psp, \
             tc.tile_pool(name="uallp", bufs=1, space="PSUM") as uallp, \
             tc.tile_pool(name="rpsp", bufs=1, space="PSUM") as rpsp, \
             tc.tile_pool(name="rp", bufs=1) as rp:
            iota_t = cp.tile([128, NCB * M], bf16)
            nc.sync.dma_start(iota_t[:], iota[:])
            wtile_t = cp.tile([128, QW * M], f32)
            nc.sync.dma_start(wtile_t[:], wtile[:])
            ones_t = cp.tile([128, 1], f32)
            nc.sync.dma_start(ones_t[:], ones1[:])
            g8_t = cp.tile([2 * M, 16], f32)
            nc.sync.dma_start(g8_t[:], g8[:])
            code_t = cp.tile([128, nch], f32)
            nc.sync.dma_start(code_t[:], code[:])

            u_all = uallp.tile([2 * M, npair], f32)

            from contextlib import ExitStack as _ES
            with (tc.For_i(0, nrep, 1) if nrep > 1 else _ES()):
                for g0 in range(0, nch, NCB):
                    xh_t = xhp.tile([128, NCB * 128], bf16, tag="xh")
                    nc.sync.dma_start(xh_t[:], xh[:, g0 * 128:(g0 + NCB) * 128])
                    oh = ohp.tile([128, NCB * M], bf16, tag="oh")
                    cb = code_t[:, g0:g0 + NCB].unsqueeze(2).broadcast_to([128, NCB, M])
                    nc.vector.tensor_tensor(oh[:].rearrange("p (c m) -> p c m", c=NCB),
                                            iota_t[:].rearrange("p (c m) -> p c m", c=NCB),
                                            cb, mybir.AluOpType.is_equal)
                    for ql in range(NCB // (QW * F)):
                        ps4 = psp.tile([128, QW * M], f32, tag="ps")
                        for h in range(QW):
                            for c in range(F):
                                lc = ql * QW * F + h * F + c
                                nc.tensor.matmul(
                                    ps4[:, h * M:(h + 1) * M],
                                    xh_t[:, lc * 128:(lc + 1) * 128],
                                    oh[:, lc * M:(lc + 1) * M],
                                    start=(c == 0), stop=(c == F - 1))
                        u4 = usbp.tile([128, QW * M], f32, tag="u")
                        nc.vector.tensor_tensor(u4[:], ps4[:], wtile_t[:],
                                                mybir.AluOpType.mult)
                        quad = g0 // (QW * F) + ql
                        for hp in range(QW // 2):
                            pair = quad * (QW // 2) + hp
                            nc.tensor.matmul(u_all[:, pair:pair + 1],
                                             u4[:, hp * 2 * M:(hp + 1) * 2 * M],
                                             ones_t[:], start=True, stop=True)

                uall_sb = rp.tile([2 * M, npair], f32)
                nc.scalar.copy(uall_sb[:], u_all[:])
                NR = 512
                for p0 in range(0, npair, NR):
                    pn = min(NR, npair - p0)
                    rps = rpsp.tile([16, pn], f32, tag="rps")
                    nc.tensor.matmul(rps[:], g8_t[:], uall_sb[:, p0:p0 + pn],
                                     start=True, stop=True)
                    rsb = rp.tile([16, pn], f32, tag="rsb")
                    nc.vector.tensor_copy(rsb[:], rps[:])
                    nc.sync.dma_start(r[:, p0:p0 + pn], rsb[:])
    nc.compile()
    return nc


def _host_prep(M, F, NCB, QW, X, W, central_species, structural_indices,
               n_cores=N_CORES, check_only=False):
    SW = M // N_SPECIES
    N = X.shape[0]
    A = N // n_cores
    assert A * n_cores == N
    nch_real = (A + 127) // 128
    nch = ((nch_real + NCB - 1) // NCB) * NCB
    Apad = nch * 128
    nwin = nch // F

    code_all = structural_indices.astype(np.int64) * N_SPECIES + central_species

    bases_all, codes = [], []
    for c in range(n_cores):
        sl = slice(c * A, (c + 1) * A)
        g_c = structural_indices[sl]
        first_idx = np.arange(nwin) * (F * 128)
        first_idx_real = np.minimum(first_idx, A - 1)
        bases = g_c[first_idx_real].astype(np.int64)
        bases[first_idx >= A] = 0
        code_c = code_all[sl] - np.repeat(bases, F * 128)[:A] * N_SPECIES
        if code_c.min() < 0 or code_c.max() >= M:
            return None  # window span violated -> caller degrades F/M
        bases_all.append(bases)
        codes.append(code_c)
    if check_only:
        return True

    Xhi = X.astype(ml_dtypes.bfloat16)
    Xlo = (X - Xhi.astype(np.float32)).astype(ml_dtypes.bfloat16)

    iota_np = np.tile(np.arange(M, dtype=np.float32), (128, NCB)).astype(ml_dtypes.bfloat16)
    ones_np = np.ones((128, 1), np.float32)
    g8_np = np.zeros((2 * M, 16), np.float32)
    for b in range(2):
        for q in range(SW):
            for s in range(N_SPECIES):
                g8_np[M * b + N_SPECIES * q + s, SW * b + q] = 1.0
    wt = W[:, :, 0]
    wcol = np.concatenate([wt.T, wt.T], axis=0)
    wtile_np = np.ascontiguousarray(np.tile(wcol, (1, QW * SW)).astype(np.float32))

    in_maps = []
    for c in range(n_cores):
        sl = slice(c * A, (c + 1) * A)
        code_pad = np.zeros(Apad, np.float32)
        code_pad[:A] = codes[c].astype(np.float32)
        code_np = np.ascontiguousarray(code_pad.reshape(nch, 128).T)
        xhl = np.zeros((Apad, 128), ml_dtypes.bfloat16)
        xhl[:A, :D_FEAT] = Xhi[sl]
        xhl[:A, D_FEAT:] = Xlo[sl]
        xh_np = np.ascontiguousarray(
            xhl.reshape(nch, 128, 128).transpose(1, 0, 2).reshape(128, nch * 128))
        in_maps.append({
            "xh": xh_np, "code": code_np, "iota": iota_np, "wtile": wtile_np,
            "ones1": ones_np, "g8": g8_np,
        })
    return in_maps, bases_all, nch, nwin


def _host_merge(M, r_list, bases_all, n_structures):
    SW = M // N_SPECIES
    out = np.zeros(n_structures, np.float64)
    for r, bases in zip(r_list, bases_all):
        npair = r.shape[1]
        for b in range(2):
            w_idx = 2 * np.arange(npair) + b
            idx = (bases[w_idx][:, None] + np.arange(SW)[None, :]).ravel()
            vals = r[SW * b:SW * b + SW, :].T.ravel().astype(np.float64)
            ok = idx < n_structures
            np.add.at(out, idx[ok], vals[ok])
    return out.astype(np.float32)[:, None]


# (M, F, NCB, QW) ladder: first whose window-span check passes is used.
PARAM_LADDER = [
    (48, 3, 48, 4),
    (64, 4, 64, 4),
    (128, 8, 32, 2),
]


def _get_compiled(params, nch, nrep=1):
    key = (params, nch, nrep)
    if key not in _cache:
        M, F, NCB, QW = params
        _cache[key] = _build_program(M, F, NCB, QW, nch, nrep=nrep)
    return _cache[key]


def _timing_setup(inputs):
    X = np.ascontiguousarray(np.asarray(inputs["X"], dtype=np.float32))
    W = np.asarray(inputs["W"], dtype=np.float32)
    sp = np.asarray(inputs["central_species"]).astype(np.int64)
    g = np.asarray(inputs["structural_indices"]).astype(np.int64)
    params = None
    for cand in PARAM_LADDER:
        M, F, NCB, QW = cand
        if _host_prep(M, F, NCB, QW, X, W, sp, g, check_only=True):
            params = cand
            break
    assert params is not None
    M, F, NCB, QW = params
    in_maps, bases_all, nch, nwin = _host_prep(M, F, NCB, QW, X, W, sp, g)

    def build(nrep):
        return _get_compiled(params, nch, nrep=nrep)

    return build, in_maps


def kernel(X, W, central_species, structural_indices, n_structures):
    from concourse.bass_utils import run_bass_kernel_spmd

    X = np.ascontiguousarray(np.asarray(X, dtype=np.float32))
    W = np.asarray(W, dtype=np.float32)
    central_species = np.asarray(central_species).astype(np.int64)
    structural_indices = np.asarray(structural_indices).astype(np.int64)
    n_structures = int(np.asarray(n_structures))

    params = None
    for cand in PARAM_LADDER:
        M, F, NCB, QW = cand
        if _host_prep(M, F, NCB, QW, X, W, central_species, structural_indices,
                      check_only=True):
            params = cand
            break
    assert params is not None, "no window parameterization fits this data"
    M, F, NCB, QW = params

    in_maps, bases_all, nch, nwin = _host_prep(M, F, NCB, QW, X, W,
                                               central_species, structural_indices)
    nc = _get_compiled(params, nch)
    res = run_bass_kernel_spmd(nc, in_maps, list(range(N_CORES)))
    out = _host_merge(M, [res.results[c]["r"] for c in range(N_CORES)],
                      bases_all, n_structures)
    return out



# revision 4
# speedup vs baseline: 1.0089x; 1.0089x over previous
"""Trainium2 Bass kernel v2 for nn_Atomistic (per-species linear + segment sum).

Math:  out[j] = sum_{atoms a with structural_indices[a]==j} X[a,:] @ W[species[a],:,0]

v2 strategy (vs the hi/lo-split baseline):
  * X ships as bf16 only (rel err ~0.4% << 2e-2 gate) -> 32.5 MB/core HBM
    traffic, the memory roofline.
  * Matmul orientation flipped: the one-hot oh[a, m] (M = 8*SW codes,
    code = 8*(g - window_base) + species) is the STATIONARY operand
    (LDWEIGHTS P=M=32 is ~2x cheaper than P=128), X chunks stream as the
    moving operand.  PS[m, d] += oh.T @ xh accumulates F chunks per window.
  * npack windows pack along PSUM partitions at tile_position col-group
    offsets (0/32/64/96), 8 windows along columns: one 2 KB PSUM bank holds
    npack*8 window accumulators = one "fill" of CPF chunks.
  * Flush: ACT (idle otherwise) copies PSUM -> SBUF bf16; DVE multiplies by
    the replicated W tile (all-bf16 = 2x rate) and block-reduces over d into
    a persistent [128, nfill*8] accumulator; one output DMA at the end.
  * Host: index prep, bf16 cast, layout, final (window, q, species) fold.
"""
import sys

sys.path.insert(0, "/opt/trn_rl_repo")

import numpy as np
import ml_dtypes

N_ATOMS = 2_000_000
D_FEAT = 64
OUT_DIM = 1
N_SPECIES = 8
N_STRUCTURES = 20_000
N_CORES = 8

f32 = None
bf16 = None
_cache = {}


def _imports():
    global f32, bf16
    import concourse.mybir as mybir
    f32 = mybir.dt.float32
    bf16 = mybir.dt.bfloat16


NBANKS = 2                       # PSUM banks per fill (amortizes DVE op cost)


def _geom(SW, F):
    M = 8 * SW
    # PE quadrant 3 (partition base 96) is unusable -> at most 3 col-group
    # slots at 32-aligned offsets.  Rows [M, poff) of each block stay
    # stale/unwritten in PSUM; the host merge never reads them.
    poff = ((M + 31) // 32) * 32
    npack = min(128 // poff, 3)
    NB = 8 * NBANKS              # 64-col window blocks per fill
    CPF = npack * NB * F         # chunks per PSUM fill
    return M, npack, poff, CPF, NB


DG = 1  # fills per DMA (CPF=48 already gives 6KB contiguous runs/partition)
OH_ON_POOL = False  # build the one-hot on GPSIMD, freeing the DVE for flushes


def _build_program(SW, F, nch, nrep=1, n_cores=N_CORES,
                   xh_bufs=4, oh_bufs=3, ps_bufs=3, s_bufs=3, u_bufs=3):
    import concourse.mybir as mybir
    from concourse import tile, bacc
    _imports()
    M, npack, poff, CPF, NB = _geom(SW, F)
    PSC = NB * 64                # PSUM cols per fill
    assert nch % (CPF * DG) == 0
    nfill = nch // CPF
    nc = bacc.Bacc("TRN2", target_bir_lowering=False, debug=False,
                   num_devices=n_cores)
    xh = nc.dram_tensor("xh", [128, nch * 64], bf16, kind="ExternalInput").ap()
    code = nc.dram_tensor("code", [128, nch], bf16, kind="ExternalInput").ap()
    iota = nc.dram_tensor("iota", [128, CPF * M], bf16, kind="ExternalInput").ap()
    wtile = nc.dram_tensor("wtile", [128, PSC], bf16, kind="ExternalInput").ap()
    r = nc.dram_tensor("r", [128, nfill * NB], f32, kind="ExternalOutput").ap()

    OH_ENG = nc.gpsimd if OH_ON_POOL else nc.vector
    with tile.TileContext(nc) as tc:
        with tc.tile_pool(name="const", bufs=1) as cp, \
             tc.tile_pool(name="xhp", bufs=xh_bufs) as xhp, \
             tc.tile_pool(name="ohp", bufs=oh_bufs) as ohp, \
             tc.tile_pool(name="spp", bufs=s_bufs) as spp, \
             tc.tile_pool(name="upp", bufs=u_bufs) as upp, \
             tc.tile_pool(name="vpp", bufs=u_bufs) as vpp, \
             tc.tile_pool(name="psp", bufs=ps_bufs, space="PSUM") as psp:
            iota_t = cp.tile([128, CPF * M], bf16)
            nc.sync.dma_start(iota_t[:], iota[:])
            wtile_t = cp.tile([128, PSC], bf16)
            nc.sync.dma_start(wtile_t[:], wtile[:])
            code_t = cp.tile([128, nch], bf16)
            nc.sync.dma_start(code_t[:], code[:])
            racc = cp.tile([128, nfill * NB], f32)

            L = npack * poff     # live PSUM partition range

            def flush(ps, f):
                # PSUM -> bf16 SBUF on the (otherwise idle) scalar engine,
                # then W-apply + blocked d-reduction on DVE at 2x bf16 rate.
                s_t = spp.tile([128, PSC], bf16, tag="s")
                nc.scalar.copy(s_t[0:L, :], ps[0:L, :])
                u = upp.tile([128, PSC], bf16, tag="u")
                nc.vector.tensor_tensor(u[0:L, :], s_t[0:L, :],
                                        wtile_t[0:L, :],
                                        mybir.AluOpType.mult)
                # First d-halving as a dense-inner TT add (2x-eligible),
                # then the 1x-only tensor_reduce runs on half the data.
                v = vpp.tile([128, PSC // 2], bf16, tag="v")
                u4 = u[0:L, :].rearrange("p (b t d) -> p b t d",
                                         b=NB, t=2)
                nc.vector.tensor_tensor(
                    v[0:L, :].rearrange("p (b o d) -> p b o d", b=NB, o=1),
                    u4[:, :, 0:1, :], u4[:, :, 1:2, :],
                    mybir.AluOpType.add)
                nc.vector.tensor_reduce(
                    racc[0:L, f * NB:(f + 1) * NB],
                    v[0:L, :].rearrange("p (b d) -> p b d", b=NB),
                    mybir.AxisListType.X, mybir.AluOpType.add)

            from contextlib import ExitStack as _ES
            with (tc.For_i(0, nrep, 1) if nrep > 1 else _ES()):
                # Software-pipelined by one fill: oh_f is emitted BEFORE the
                # flush of fill f-1 so the in-order DVE queue never blocks
                # the PE behind a flush that itself waits on the PE.
                prev = None
                for f in range(nfill):
                    if f % DG == 0:
                        xh_t = xhp.tile([128, DG * CPF * 64], bf16, tag="xh")
                        nc.sync.dma_start(
                            xh_t[:],
                            xh[:, f * CPF * 64:(f + DG) * CPF * 64])
                    xo = (f % DG) * CPF * 64
                    oh = ohp.tile([128, CPF * M], bf16, tag="oh")
                    cb = code_t[:, f * CPF:(f + 1) * CPF].unsqueeze(2) \
                        .broadcast_to([128, CPF, M])
                    OH_ENG.tensor_tensor(
                        oh[:].rearrange("p (c m) -> p c m", c=CPF),
                        iota_t[:].rearrange("p (c m) -> p c m", c=CPF),
                        cb, mybir.AluOpType.is_equal)
                    if prev is not None:
                        flush(*prev)
                    ps = psp.tile([128, PSC], f32, tag="ps")
                    for lc in range(CPF):
                        v, ph = lc // F, lc % F
                        b, rr = v // npack, v % npack
                        nc.tensor.matmul(
                            ps[rr * poff:rr * poff + M, b * 64:(b + 1) * 64],
                            oh[:, lc * M:(lc + 1) * M],
                            xh_t[:, xo + lc * 64:xo + (lc + 1) * 64],
                            start=(ph == 0), stop=(ph == F - 1))
                    prev = (ps, f)
                flush(*prev)
            nc.sync.dma_start(r[:], racc[:])
    nc.compile()
    return nc


def _host_prep(SW, F, X, W, central_species, structural_indices,
               n_cores=N_CORES, check_only=False):
    M, npack, poff, CPF, NB = _geom(SW, F)
    N = X.shape[0]
    A = N // n_cores
    assert A * n_cores == N
    nch_real = (A + 127) // 128
    blk = CPF * DG
    nch = ((nch_real + blk - 1) // blk) * blk
    Apad = nch * 128
    WS = F * 128
    nwin = nch // F

    bases_all, codes = [], []
    for c in range(n_cores):
        sl = slice(c * A, (c + 1) * A)
        g_c = structural_indices[sl]
        first = np.arange(nwin) * WS
        fc = np.minimum(first, A - 1)
        bases = g_c[fc].astype(np.int64)
        bases[first >= A] = 0
        code_c = (g_c - np.repeat(bases, WS)[:A]) * N_SPECIES \
            + central_species[sl]
        if code_c.min() < 0 or code_c.max() >= M:
            return None  # window span violated -> caller degrades SW/F
        bases_all.append(bases)
        codes.append(code_c)
    if check_only:
        return True

    iota_np = np.tile(np.arange(M, dtype=np.float32),
                      (128, CPF)).astype(ml_dtypes.bfloat16)
    W2 = W[:, :, 0]                                   # [8, 64]
    rows = W2[(np.arange(128) % poff) % N_SPECIES]    # [128, 64]
    wtile_np = np.ascontiguousarray(
        np.tile(rows, (1, NB)).astype(ml_dtypes.bfloat16))

    in_maps = []
    for c in range(n_cores):
        sl = slice(c * A, (c + 1) * A)
        code_pad = np.full(Apad, -1.0, np.float32)
        code_pad[:A] = codes[c].astype(np.float32)
        code_np = np.ascontiguousarray(
            code_pad.reshape(nch, 128).T).astype(ml_dtypes.bfloat16)
        xhl = np.zeros((Apad, D_FEAT), ml_dtypes.bfloat16)
        xhl[:A] = X[sl].astype(ml_dtypes.bfloat16)
        xh_np = np.ascontiguousarray(
            xhl.reshape(nch, 128, D_FEAT).transpose(1, 0, 2)
               .reshape(128, nch * D_FEAT))
        in_maps.append({"xh": xh_np, "code": code_np, "iota": iota_np,
                        "wtile": wtile_np})
    return in_maps, bases_all, nch


def _host_merge(SW, F, r_list, bases_all, n_structures):
    M, npack, poff, CPF, NB = _geom(SW, F)
    out = np.zeros(n_structures + SW, np.float64)
    # partition p = rr*poff + q*8 + s  ->  gather valid rows
    P_idx = (np.arange(npack)[:, None, None] * poff
             + np.arange(SW)[None, :, None] * 8
             + np.arange(8)[None, None, :])          # [npack, SW, 8]
    qs = np.arange(SW)
    for r, bases in zip(r_list, bases_all):
        nfill = r.shape[1] // NB
        Rv = r[P_idx.reshape(-1), :].reshape(npack, SW, 8, nfill, NB)
        T = Rv.astype(np.float64).sum(axis=2)        # [npack, SW, nfill, NB]
        # window w = f*(NB*npack) + b*npack + rr
        vals = T.transpose(2, 3, 0, 1).reshape(-1, SW)   # [nwin, SW]
        seg = np.minimum(bases[:, None] + qs[None, :], n_structures + SW - 1)
        out += np.bincount(seg.ravel(), weights=vals.ravel(),
                           minlength=n_structures + SW)
    return out[:n_structures].astype(np.float32)[:, None]


# (SW, F) ladder: first whose window-span check passes is used.
# (M = 8*SW must equal the col-group offset -> SW in {4, 8} only.)
PARAM_LADDER = [
    (3, 1),
    (4, 1),
    (8, 4),
]


def _get_compiled(params, nch, nrep=1):
    key = (params, nch, nrep)
    if key not in _cache:
        SW, F = params
        _cache[key] = _build_program(SW, F, nch, nrep=nrep)
    return _cache[key]


def _plan(X, W, central_species, structural_indices):
    X = np.ascontiguousarray(np.asarray(X, dtype=np.float32))
    W = np.asarray(W, dtype=np.float32)
    central_species = np.asarray(central_species).astype(np.int64)
    structural_indices = np.asarray(structural_indices).astype(np.int64)
    params = None
    for cand in PARAM_LADDER:
        SW, F = cand
        if _host_prep(SW, F, X, W, central_species, structural_indices,
                      check_only=True):
            params = cand
            break
    assert params is not None, "no window parameterization fits this data"
    SW, F = params
    in_maps, bases_all, nch = _host_prep(SW, F, X, W, central_species,
                                         structural_indices)
    return params, in_maps, bases_all, nch


def _timing_setup(inputs):
    params, in_maps, bases_all, nch = _plan(
        inputs["X"], inputs["W"], inputs["central_species"],
        inputs["structural_indices"])

    def build(nrep):
        return _get_compiled(params, nch, nrep=nrep)

    return build, in_maps


def kernel(X, W, central_species, structural_indices, n_structures):
    from concourse.bass_utils import run_bass_kernel_spmd

    n_structures = int(np.asarray(n_structures))
    params, in_maps, bases_all, nch = _plan(X, W, central_species,
                                            structural_indices)
    nc = _get_compiled(params, nch)
    res = run_bass_kernel_spmd(nc, in_maps, list(range(N_CORES)))
    SW, F = params
    out = _host_merge(SW, F, [res.results[c]["r"] for c in range(N_CORES)],
                      bases_all, n_structures)
    return out
